# revision 5
# baseline (speedup 1.0000x reference)
"""Trainium2 Bass kernel for nn_CalibrationLoss (histogram-binning calibration loss).

Full inputs: logits [32768, 1000] f32, labels [32768] int64 -> scalar f32 loss.

Strategy (data-parallel over batch, 8 cores x 4096 rows):
  Per core, per row r:
    negmx[r]  = -max_c logits[r, c]                  (DVE reduce, negate)
    denom[r]  = sum_c exp(logits[r, c] - max)        (ACT Exp with fused accum)
    conf[r]   = 1 / denom[r]                         (= max softmax prob)
    corr[r]   = (logits[r, label[r]] == max)         (is_equal on host-gathered
                                                      label logit, exact f32)
  Then 15 soft bins: in_bin = sig(20*(conf-lo)) * sig(20*(hi-conf)) and three
  per-bin partial sums (mass, conf, correct) accumulated with fused
  tensor_tensor_reduce into a [128, 45] stats tile, reduced over partitions via
  a ones-vector matmul to [1, 45], DMA'd out. Host sums the 8x45 scalars and
  applies the final (trivial) combine.
"""

import numpy as np

import concourse.bass as bass
import concourse.tile as tile
from concourse import bacc, mybir
from concourse.bass import get_trn_type
from concourse.bass_utils import run_bass_kernel_spmd

N_BINS = 15
SHARPNESS = 20.0
EPS = 1e-8

B, C = 32768, 1000
N_CORES = 8
ROWS = B // N_CORES      # 4096 rows per core
P = 128                  # SBUF partitions
ROWS_PER_DMA = 2         # 128-row groups per DMA chunk (1 MB chunks)

FP32 = mybir.dt.float32


def build_nc(rows=ROWS, rows_per_dma=ROWS_PER_DMA, big_bufs=3, scratch_bufs=2):
    """Build + compile the per-core Bass module.

    Per-core inputs:
      logits [rows, C] f32
      neglab [128, rows//128] f32, neglab[p, t] = -logits[t*128+p, label]
    Per-core output:
      out [1, 45] f32 = per-bin [mass, conf_sum, correct_sum]
    """
    assert rows % (P * rows_per_dma) == 0
    n_chunks = rows // (P * rows_per_dma)
    nt = rows // P           # per-row stats live in [128, nt]

    nc = bacc.Bacc(get_trn_type() or "TRN2", target_bir_lowering=False, debug=False)

    logits = nc.dram_tensor("logits", [rows, C], FP32, kind="ExternalInput")
    neglab = nc.dram_tensor("neglab", [P, nt], FP32, kind="ExternalInput")
    out = nc.dram_tensor("out", [1, 3 * N_BINS], FP32, kind="ExternalOutput")

    with tile.TileContext(nc) as tc:
        with (
            tc.tile_pool(name="big", bufs=big_bufs) as big,
            tc.tile_pool(name="scratch", bufs=scratch_bufs) as scratch,
            tc.tile_pool(name="stats", bufs=1) as stp,
            tc.tile_pool(name="small", bufs=2) as small,
            tc.tile_pool(name="psum", bufs=1, space=bass.MemorySpace.PSUM) as psp,
        ):
            negmx = stp.tile([P, nt], FP32)
            denom = stp.tile([P, nt], FP32)
            conf = stp.tile([P, nt], FP32)
            corr = stp.tile([P, nt], FP32)
            neglab_sb = stp.tile([P, nt], FP32)
            stats = stp.tile([P, 3 * N_BINS], FP32)
            ones = stp.tile([P, 1], FP32)
            biases = stp.tile([P, 2 * N_BINS], FP32)

            nc.sync.dma_start(out=neglab_sb, in_=neglab.ap())
            nc.vector.memset(ones, 1.0)
            bnds = np.linspace(0.0, 1.0, N_BINS + 1)
            for b in range(N_BINS):
                nc.gpsimd.memset(biases[:, b : b + 1], -SHARPNESS * float(bnds[b]))
                nc.gpsimd.memset(
                    biases[:, N_BINS + b : N_BINS + b + 1],
                    SHARPNESS * float(bnds[b + 1]),
                )

            lg = logits.ap().rearrange("(k j p) c -> k p j c", p=P, j=rows_per_dma)
            for k in range(n_chunks):
                lt = big.tile([P, rows_per_dma, C], FP32)
                nc.sync.dma_start(out=lt, in_=lg[k])
                nc.vector.tensor_reduce(
                    out=negmx[:, k * rows_per_dma : (k + 1) * rows_per_dma],
                    in_=lt,
                    axis=mybir.AxisListType.X,
                    op=mybir.AluOpType.max,
                    negate=True,
                )
                for j in range(rows_per_dma):
                    t = k * rows_per_dma + j
                    ex = scratch.tile([P, C], FP32)
                    nc.scalar.activation(
                        out=ex,
                        in_=lt[:, j, :],
                        func=mybir.ActivationFunctionType.Exp,
                        bias=negmx[:, t : t + 1],
                        scale=1.0,
                        accum_out=denom[:, t : t + 1],
                    )

            nc.vector.reciprocal(out=conf, in_=denom)
            nc.vector.tensor_tensor(
                out=corr, in0=neglab_sb, in1=negmx, op=mybir.AluOpType.is_equal
            )

            for b in range(N_BINS):
                t1 = small.tile([P, nt], FP32, tag="t1")
                t2 = small.tile([P, nt], FP32, tag="t2")
                ib = small.tile([P, nt], FP32, tag="ib")
                s1 = small.tile([P, nt], FP32, tag="s1")
                s2 = small.tile([P, nt], FP32, tag="s2")
                nc.scalar.activation(
                    out=t1,
                    in_=conf,
                    func=mybir.ActivationFunctionType.Sigmoid,
                    bias=biases[:, b : b + 1],
                    scale=SHARPNESS,
                )
                nc.scalar.activation(
                    out=t2,
                    in_=conf,
                    func=mybir.ActivationFunctionType.Sigmoid,
                    bias=biases[:, N_BINS + b : N_BINS + b + 1],
                    scale=-SHARPNESS,
                )
                # NOTE: fused tensor_tensor_reduce crashes the exec unit on
                # this HW/runtime combo (NRT_EXEC_UNIT_UNRECOVERABLE) — use
                # unfused mul + reduce instead.
                nc.vector.tensor_mul(ib, t1, t2)
                nc.vector.tensor_reduce(
                    out=stats[:, b : b + 1],
                    in_=ib,
                    axis=mybir.AxisListType.X,
                    op=mybir.AluOpType.add,
                )
                nc.vector.tensor_mul(s1, ib, conf)
                nc.vector.tensor_reduce(
                    out=stats[:, N_BINS + b : N_BINS + b + 1],
                    in_=s1,
                    axis=mybir.AxisListType.X,
                    op=mybir.AluOpType.add,
                )
                nc.vector.tensor_mul(s2, ib, corr)
                nc.vector.tensor_reduce(
                    out=stats[:, 2 * N_BINS + b : 2 * N_BINS + b + 1],
                    in_=s2,
                    axis=mybir.AxisListType.X,
                    op=mybir.AluOpType.add,
                )

            ps = psp.tile([1, 3 * N_BINS], FP32)
            nc.tensor.matmul(ps, ones, stats, start=True, stop=True)
            osb = stp.tile([1, 3 * N_BINS], FP32)
            nc.scalar.copy(out=osb, in_=ps)
            nc.sync.dma_start(out=out.ap(), in_=osb)

    nc.compile()
    return nc


_NC_CACHE = {}


def _get_nc():
    if "nc" not in _NC_CACHE:
        _NC_CACHE["nc"] = build_nc()
    return _NC_CACHE["nc"]


def make_in_maps(logits, labels):
    """Shard full inputs into per-core input maps."""
    logits = np.ascontiguousarray(np.asarray(logits, dtype=np.float32))
    labels = np.asarray(labels).astype(np.int64).ravel()
    assert logits.shape == (B, C), logits.shape
    assert labels.shape == (B,), labels.shape
    lab_logit = logits[np.arange(B), labels]  # [B] f32, exact gather
    nt = ROWS // P
    in_maps = []
    for c in range(N_CORES):
        sl = slice(c * ROWS, (c + 1) * ROWS)
        neglab = np.ascontiguousarray((-lab_logit[sl]).reshape(nt, P).T)
        in_maps.append({"logits": logits[sl], "neglab": neglab})
    return in_maps


def combine(per_core_stats):
    """Host combine: sum 8x[45] partials, finish the loss in float64."""
    stats = np.zeros(3 * N_BINS, dtype=np.float64)
    for s in per_core_stats:
        stats += np.asarray(s, dtype=np.float64).reshape(-1)
    mass = stats[0:N_BINS]
    csum = stats[N_BINS : 2 * N_BINS]
    asum = stats[2 * N_BINS : 3 * N_BINS]
    acc = asum / (mass + EPS)
    cnf = csum / (mass + EPS)
    loss = np.sum((acc - cnf) ** 2 * mass) / B
    return np.asarray(np.float32(loss))


def run_device(logits, labels, trace=False, **kwargs):
    in_maps = make_in_maps(logits, labels)
    res = run_bass_kernel_spmd(
        _get_nc(), in_maps, core_ids=list(range(N_CORES)), trace=trace, **kwargs
    )
    loss = combine([r["out"] for r in res.results])
    return loss, res


def kernel(logits, labels):
    loss, _ = run_device(logits, labels)
    return loss


# revision 6
# speedup vs baseline: 1.2851x; 1.2851x over previous
"""Trainium2 Bass kernel for nn_CalibrationLoss (histogram-binning calibration loss).

Full inputs: logits [32768, 1000] f32, labels [32768] int64 -> scalar f32 loss.

Strategy (data-parallel over batch, 8 cores x 4096 rows):
  Main loop over 16 x 1MB chunks ([128p, 2, 1000]):
    negmx  = -max_c logits (DVE reduce, negate)     } independent, so the ACT
    sum_c exp(logits) via ACT Exp + fused accum     } exp never waits on DVE
  conf = 1 / (denom * exp(negmx))  (= max softmax prob, since
         denom*exp(-mx) = sum exp(l-mx));  correct = (l[label] == mx) via
  is_equal on a host-gathered label logit (exact f32 compare).
  Soft-bin stats are computed per half (16 row-columns) with broadcast-AP
  tensor ops in a [p, bin, col] layout — two big sigmoids per half instead of
  30 small ones — reduced over cols on DVE, then one ones-matmul reduces
  partitions to [1, 90]; the host sums 8 x 90 scalars and finishes the loss.
"""

import numpy as np

import concourse.bass as bass
import concourse.tile as tile
from concourse import bacc, mybir
from concourse.bass import get_trn_type
from concourse.bass_utils import run_bass_kernel_spmd

N_BINS = 15
SHARPNESS = 20.0
EPS = 1e-8

B, C = 32768, 1000
N_CORES = 8
ROWS = B // N_CORES      # 4096 rows per core
P = 128                  # SBUF partitions
ROWS_PER_DMA = 2         # 128-row groups per DMA chunk (1 MB chunks)

FP32 = mybir.dt.float32


def _bcast_cols(ap_pt, n_bins):
    """[128, T] AP -> [128, n_bins, T] with a step-0 bin dim."""
    return bass.AP(
        tensor=ap_pt.tensor,
        offset=ap_pt.offset,
        ap=[ap_pt.ap[0], [0, n_bins], ap_pt.ap[1]],
    )


def _bcast_bins(ap_pb, t):
    """[128, n_bins] AP -> [128, n_bins, T] with a step-0 col dim."""
    return bass.AP(
        tensor=ap_pb.tensor,
        offset=ap_pb.offset,
        ap=[ap_pb.ap[0], ap_pb.ap[1], [0, t]],
    )


def build_nc(rows=ROWS, rows_per_dma=ROWS_PER_DMA, big_bufs=4):
    """Build + compile the per-core Bass module.

    Per-core inputs:
      logits [rows, C] f32
      neglab [128, rows//128] f32, neglab[p, t] = -logits[t*128+p, label]
    Per-core output:
      out [1, 90] f32 = two halves of per-bin [mass, conf_sum, correct_sum]
    """
    assert rows % (P * rows_per_dma * 2) == 0
    n_chunks = rows // (P * rows_per_dma)
    nt = rows // P           # per-row stats live in [128, nt]
    nh = nt // 2             # columns per half

    nc = bacc.Bacc(get_trn_type() or "TRN2", target_bir_lowering=False, debug=False)

    logits = nc.dram_tensor("logits", [rows, C], FP32, kind="ExternalInput")
    neglab = nc.dram_tensor("neglab", [P, nt], FP32, kind="ExternalInput")
    out = nc.dram_tensor("out", [1, 2 * 3 * N_BINS], FP32, kind="ExternalOutput")

    with tile.TileContext(nc) as tc:
        with (
            tc.tile_pool(name="big", bufs=big_bufs) as big,
            tc.tile_pool(name="scratch", bufs=2) as scratch,
            tc.tile_pool(name="stp", bufs=1) as stp,
            tc.tile_pool(name="small", bufs=2) as small,
            tc.tile_pool(name="psum", bufs=1, space=bass.MemorySpace.PSUM) as psp,
        ):
            negmx = stp.tile([P, nt], FP32)
            denom = stp.tile([P, nt], FP32)
            conf = stp.tile([P, nt], FP32)
            corr = stp.tile([P, nt], FP32)
            neglab_sb = stp.tile([P, nt], FP32)
            stats = stp.tile([P, 2, 3 * N_BINS], FP32)
            ones = stp.tile([P, 1], FP32)
            lo_t = stp.tile([P, N_BINS], FP32)
            hi_t = stp.tile([P, N_BINS], FP32)

            lg = logits.ap().rearrange("(k j p) c -> k p j c", p=P, j=rows_per_dma)

            def emit_chunk(k):
                lt = big.tile([P, rows_per_dma, C], FP32, tag="lt")
                nc.sync.dma_start(out=lt, in_=lg[k])
                nc.vector.tensor_reduce(
                    out=negmx[:, k * rows_per_dma : (k + 1) * rows_per_dma],
                    in_=lt,
                    axis=mybir.AxisListType.X,
                    op=mybir.AluOpType.max,
                    negate=True,
                )
                for j in range(rows_per_dma):
                    t = k * rows_per_dma + j
                    ex = scratch.tile([P, C], FP32, tag="ex")
                    nc.scalar.activation(
                        out=ex,
                        in_=lt[:, j, :],
                        func=mybir.ActivationFunctionType.Exp,
                        bias=0.0,
                        scale=1.0,
                        accum_out=denom[:, t : t + 1],
                    )

            def emit_half(h):
                sl = slice(h * nh, (h + 1) * nh)
                # conf = 1 / (denom * exp(negmx));  corr = (neglab == negmx)
                eh = small.tile([P, nh], FP32, tag="eh")
                nc.scalar.activation(
                    out=eh,
                    in_=negmx[:, sl],
                    func=mybir.ActivationFunctionType.Exp,
                    bias=0.0,
                    scale=1.0,
                )
                nc.vector.tensor_mul(eh, eh, denom[:, sl])
                nc.vector.reciprocal(out=conf[:, sl], in_=eh)
                nc.vector.tensor_tensor(
                    out=corr[:, sl],
                    in0=neglab_sb[:, sl],
                    in1=negmx[:, sl],
                    op=mybir.AluOpType.is_equal,
                )
                conf_b = _bcast_cols(conf[:, sl], N_BINS)   # [P, 15, nh]
                corr_b = _bcast_cols(corr[:, sl], N_BINS)
                a1 = small.tile([P, N_BINS, nh], FP32, tag="a1")
                a2 = small.tile([P, N_BINS, nh], FP32, tag="a2")
                ib = small.tile([P, N_BINS, nh], FP32, tag="ib")
                ww = small.tile([P, N_BINS, nh], FP32, tag="ww")
                # a1 = conf - lo[b];  a2 = hi[b] - conf
                nc.vector.tensor_tensor(
                    out=a1, in0=conf_b, in1=_bcast_bins(lo_t[:, :], nh),
                    op=mybir.AluOpType.subtract,
                )
                nc.vector.tensor_tensor(
                    out=a2, in0=_bcast_bins(hi_t[:, :], nh), in1=conf_b,
                    op=mybir.AluOpType.subtract,
                )
                # t1 = sigmoid(20*a1), t2 = sigmoid(20*a2) — two big ACT ops
                nc.scalar.activation(
                    out=a1, in_=a1, func=mybir.ActivationFunctionType.Sigmoid,
                    bias=0.0, scale=SHARPNESS,
                )
                nc.scalar.activation(
                    out=a2, in_=a2, func=mybir.ActivationFunctionType.Sigmoid,
                    bias=0.0, scale=SHARPNESS,
                )
                nc.vector.tensor_mul(ib, a1, a2)
                nc.vector.tensor_reduce(
                    out=stats[:, h, 0:N_BINS], in_=ib,
                    axis=mybir.AxisListType.X, op=mybir.AluOpType.add,
                )
                nc.vector.tensor_mul(ww, ib, conf_b)
                nc.vector.tensor_reduce(
                    out=stats[:, h, N_BINS : 2 * N_BINS], in_=ww,
                    axis=mybir.AxisListType.X, op=mybir.AluOpType.add,
                )
                nc.vector.tensor_mul(ww, ib, corr_b)
                nc.vector.tensor_reduce(
                    out=stats[:, h, 2 * N_BINS : 3 * N_BINS], in_=ww,
                    axis=mybir.AxisListType.X, op=mybir.AluOpType.add,
                )

            # main loop, with per-half stats processing interleaved so only
            # the second half's (small) tail runs after the last chunk
            for k in range(n_chunks // 2):
                emit_chunk(k)
            # constants + neglab load (off the critical DMA path)
            nc.sync.dma_start(out=neglab_sb, in_=neglab.ap())
            nc.vector.memset(ones, 1.0)
            bnds = np.linspace(0.0, 1.0, N_BINS + 1)
            for b in range(N_BINS):
                nc.gpsimd.memset(lo_t[:, b : b + 1], float(bnds[b]))
                nc.gpsimd.memset(hi_t[:, b : b + 1], float(bnds[b + 1]))
            emit_half(0)
            for k in range(n_chunks // 2, n_chunks):
                emit_chunk(k)
            emit_half(1)

            ps = psp.tile([1, 2 * 3 * N_BINS], FP32)
            nc.tensor.matmul(
                ps, ones, stats.rearrange("p h c -> p (h c)"), start=True, stop=True
            )
            osb = stp.tile([1, 2 * 3 * N_BINS], FP32)
            nc.scalar.copy(out=osb, in_=ps)
            nc.sync.dma_start(out=out.ap(), in_=osb)

    nc.compile()
    return nc


_NC_CACHE = {}


def _get_nc():
    if "nc" not in _NC_CACHE:
        _NC_CACHE["nc"] = build_nc()
    return _NC_CACHE["nc"]


def make_in_maps(logits, labels):
    """Shard full inputs into per-core input maps."""
    logits = np.ascontiguousarray(np.asarray(logits, dtype=np.float32))
    labels = np.asarray(labels).astype(np.int64).ravel()
    assert logits.shape == (B, C), logits.shape
    assert labels.shape == (B,), labels.shape
    lab_logit = logits[np.arange(B), labels]  # [B] f32, exact gather
    nt = ROWS // P
    in_maps = []
    for c in range(N_CORES):
        sl = slice(c * ROWS, (c + 1) * ROWS)
        neglab = np.ascontiguousarray((-lab_logit[sl]).reshape(nt, P).T)
        in_maps.append({"logits": logits[sl], "neglab": neglab})
    return in_maps


def combine(per_core_stats):
    """Host combine: sum the per-core/per-half [90] partials, finish in f64."""
    stats = np.zeros(3 * N_BINS, dtype=np.float64)
    for s in per_core_stats:
        s = np.asarray(s, dtype=np.float64).reshape(2, 3 * N_BINS)
        stats += s[0] + s[1]
    mass = stats[0:N_BINS]
    csum = stats[N_BINS : 2 * N_BINS]
    asum = stats[2 * N_BINS : 3 * N_BINS]
    acc = asum / (mass + EPS)
    cnf = csum / (mass + EPS)
    loss = np.sum((acc - cnf) ** 2 * mass) / B
    return np.asarray(np.float32(loss))


def run_device(logits, labels, trace=False, **kwargs):
    in_maps = make_in_maps(logits, labels)
    res = run_bass_kernel_spmd(
        _get_nc(), in_maps, core_ids=list(range(N_CORES)), trace=trace, **kwargs
    )
    loss = combine([r["out"] for r in res.results])
    return loss, res


def kernel(logits, labels):
    loss, _ = run_device(logits, labels)
    return loss
